# revision 49
# baseline (speedup 1.0000x reference)
"""Trainium2 Bass kernel for nn_CrossAttention (B=2, Nq=Nk=2048, H=8, Dh=64,
Dx=512, Dctx=768).

Sharding: (batch, head-pair) across 8 cores — core c = (batch c//4, head pair
c%4). Each core computes K/V/Q projections for its 2 heads only (no duplicated
projection work), full attention over all 2048 queries x 2048 keys for those
heads, and a PARTIAL output projection (rows hp*128:(hp+1)*128 of Wo). The
host sums the 4 partial outputs per batch at gather time (row-parallel Wo).

Every matmul is emitted in 128x128 PE tile mode (stationary partition dim and
moving partition dim are always 128) so the PE never switches tiling modes:
mode switches drain the array and hold the clock at the 1.2GHz mid p-state.
QK keeps 128-mode by zero-padding K along the contraction (dh) axis: ktA has
K_A^T in partitions 0:64 and zeros in 64:128; ktB the reverse. The shared
moving operand qt holds Q_A^T on partitions 0:64 and Q_B^T on 64:128.

Softmax runs without max-subtraction (scores ~N(0,1)); denominators come from
a ones-column in the augmented V stationary (head A: [64 V | ones] -> attn on
psum partitions 0:64, denom on 64; head B: [zeros | ones@32 | zeros | 64 V]
-> attn on partitions 64:128, denom on 32, keeping every operand pair of the
normalize path base-partition aligned). Reciprocal is exp(-ln(d)) on the
scalar engine (one pinned activation table holds both funcs); the reciprocal
rows are broadcast across partitions with a 128-mode matmul against a
constant indicator tile, then applied on the vector engine in SBUF/bf16.

DMA notes: wall time scales with partition-row descriptors, so weights are
host-packed into wide-row tiles and the bias is added on the host at gather;
ctx streams in kv-halves so K-proj/attention start after half the bytes;
non-critical transfers are issue-gated (tiny WAW copies) so the scheduler
cannot let them steal bandwidth or front-run the attention pipeline.
"""

import sys

sys.path.insert(0, "/opt/trn_rl_repo")

import numpy as np
import ml_dtypes

import concourse.bacc as bacc
import concourse.mybir as mybir
import concourse.tile as tile
from concourse.bass_utils import run_bass_kernel_spmd
from contextlib import ExitStack

F32 = mybir.dt.float32
BF16 = mybir.dt.bfloat16
NP_BF16 = np.dtype(ml_dtypes.bfloat16)

B = 2
NQ = 2048
NKV = 2048
DX = 512
DC = 768
DI = 512
NH = 8
DH = 64
N_CORES = 8

KC_X = DX // 128  # 4 contraction chunks for x
KC_C = DC // 128  # 6 contraction chunks for context
MO = DI // 128  # 4 output-row chunks
NKC = NKV // 128  # 16 kv chunks
NQC = NQ // 512  # 4 q chunks of 512
NPAIR = NKC // 2  # 8 kv chunk-pairs per q chunk
SCALE = DH ** -0.5

_CACHE = {}


def _build_nc():
    nc = bacc.Bacc("TRN2", target_bir_lowering=False, debug=False, num_devices=N_CORES)

    # weights arrive host-packed into wide-row tiles (one DMA descriptor per
    # partition row is the DMA cost unit, so [128, K*128] beats K x [128,128])
    xt = nc.declare_dram_parameter("xt", [DX, NQ], BF16, isOutput=False)
    ctxt = nc.declare_dram_parameter("ctxt", [DC, NKV], BF16, isOutput=False)
    wq = nc.declare_dram_parameter("wq", [128, KC_X * 128], BF16, isOutput=False)
    wk = nc.declare_dram_parameter("wk", [128, KC_C * 128], BF16, isOutput=False)
    wv = nc.declare_dram_parameter("wv", [128, KC_C * 128], BF16, isOutput=False)
    wo = nc.declare_dram_parameter("wo", [128, DI], BF16, isOutput=False)
    ot = nc.declare_dram_parameter("ot", [DI, NQ], F32, isOutput=True)

    with tile.TileContext(nc) as tc:
        with ExitStack() as ctx:
            # ---- SBUF pools ----
            const_p = ctx.enter_context(tc.tile_pool(name="const", bufs=1))
            w_p = ctx.enter_context(tc.tile_pool(name="weights", bufs=1))
            ctx_p = ctx.enter_context(tc.tile_pool(name="ctxt", bufs=1))
            xt_p = ctx.enter_context(tc.tile_pool(name="xt", bufs=1))
            kt_p = ctx.enter_context(tc.tile_pool(name="kt", bufs=1))
            qt_p = ctx.enter_context(tc.tile_pool(name="qt", bufs=1))
            vaug_p = ctx.enter_context(tc.tile_pool(name="vaug", bufs=1))
            p_p = ctx.enter_context(tc.tile_pool(name="pp", bufs=3))
            at_p = ctx.enter_context(tc.tile_pool(name="at", bufs=2))
            rec_p = ctx.enter_context(tc.tile_pool(name="rec", bufs=1))
            out_p = ctx.enter_context(tc.tile_pool(name="outsb", bufs=4))
            # ---- PSUM pools: 2 + 4 + 2 = 8 banks exactly ----
            acc_ps = ctx.enter_context(tc.tile_pool(name="acc_ps", bufs=2, space="PSUM"))
            s_ps = ctx.enter_context(tc.tile_pool(name="s_ps", bufs=1, space="PSUM"))
            attn_ps = ctx.enter_context(tc.tile_pool(name="attn_ps", bufs=1, space="PSUM"))

            # Pin the activation table to natural_log_exp_and_others (id 6):
            # it serves both Exp and Ln, so the table-load pass has no reason
            # to ping-pong between exp_and_others and natural_log (13 reloads
            # x 1283ns on the bottleneck scalar engine otherwise).
            nc.scalar.add_instruction(
                mybir.InstLoadActFuncSet(
                    name=nc.get_next_instruction_name(),
                    act_func_set_id=6, ins=[], outs=[]))

            # ---- constants ----
            # e_tile: bcast selector. col c<64 -> 1.0 at partition 64 (rec_A
            # row); col c>=64 -> 1.0 at partition 32 (rec_B row). Partition
            # bases must be 32-aligned (BIR verifier).
            e_tile = const_p.tile([128, 128], BF16)
            nc.any.memset(e_tile[:], 0.0)
            nc.any.memset(e_tile[64:65, 0:64], 1.0)
            nc.any.memset(e_tile[32:33, 64:128], 1.0)
            zbias = const_p.tile([128, 1], F32)
            nc.any.memset(zbias[:], 0.0)

            # rec: reciprocal rows (bf16), zeroed once; partitions 64/96 get
            # 1/d per qc column slice. All other rows stay 0 so the bcast
            # matmul's unused partitions contribute nothing (no NaN*0 risk).
            rec = rec_p.tile([128, NQ], BF16, name="rec")
            nc.any.memset(rec[:], 0.0)
            scr = rec_p.tile([128, 512], F32, name="lnscr")

            # ---- DMA inputs ----
            # DMA wall time scales with descriptor (partition-row) count, so:
            # packed weights (128 rows each), whole ctx/x tiles (4KB rows),
            # no bias transfer (host adds bo at gather). K-proj operands lead.
            wk_sb = w_p.tile([128, KC_C * 128], BF16, name="wk_sb")
            nc.sync.dma_start(wk_sb[:], wk[:, :])
            wq_sb = w_p.tile([128, KC_X * 128], BF16, name="wq_sb")
            nc.sync.dma_start(wq_sb[:], wq[:, :])
            ctx_t = [ctx_p.tile([128, NKV], BF16, tag=f"ctx{c}", name=f"ctx{c}")
                     for c in range(KC_C)]
            xt_t = [xt_p.tile([128, NQ], BF16, tag=f"xt{c}", name=f"xt{c}")
                    for c in range(KC_X)]
            wv_sb = w_p.tile([128, KC_C * 128], BF16, name="wv_sb")
            wo_sb = w_p.tile([128, DI], BF16, name="wo_sb")
            # ctx goes in kv-halves: K-proj groups 0/1 (and attention pairs
            # 0..3) only need kv 0:1024 of every feature chunk, so attention
            # starts after half the ctx bytes. xt0/wv ride between the
            # halves; the rest is issue-gated behind ctx (see below) so it
            # cannot steal bandwidth from the critical stream.
            # big transfers are split by partition halves/quarters: same
            # descriptor count, but they spread across queues (and the tile
            # scheduler's per-queue DMA model then sees them land early, so
            # it doesn't deprioritize the consumers)
            for g in range(4):
                nc.sync.dma_start(xt_t[0][g * 32:(g + 1) * 32, :],
                                  xt[g * 32:(g + 1) * 32, :])
            for c in range(KC_C):
                for g in range(2):
                    nc.sync.dma_start(
                        ctx_t[c][g * 64:(g + 1) * 64, 0:1024],
                        ctxt[c * 128 + g * 64:c * 128 + (g + 1) * 64, 0:1024])
            for c in range(KC_C):
                for g in range(2):
                    nc.sync.dma_start(
                        ctx_t[c][g * 64:(g + 1) * 64, 1024:2048],
                        ctxt[c * 128 + g * 64:c * 128 + (g + 1) * 64, 1024:2048])
            # (wv DMA is issue-gated on the first exp output, emitted inside
            # the qc0 loop: V-proj then cannot be scheduled ahead of the
            # attention start; late PV accumulation is absorbed by PE slack
            # and the 3-deep P pool.)
            # gate: a 1-element copy from the tail of the last ctx transfer
            # into each remaining destination tile makes its DMA enqueue only
            # after ctx has fully landed (WAW through the tile dep tracker).
            for c in range(1, KC_X):
                nc.gpsimd.tensor_copy(xt_t[c][0:1, 0:1],
                                      ctx_t[KC_C - 1][0:1, 2047:2048])
                nc.sync.dma_start(xt_t[c][:], xt[c * 128:(c + 1) * 128, :])
            nc.gpsimd.tensor_copy(wo_sb[0:1, 0:1],
                                  ctx_t[KC_C - 1][0:1, 2047:2048])
            nc.sync.dma_start(wo_sb[:], wo[:, :])
            wk_t = [wk_sb[:, c * 128:(c + 1) * 128] for c in range(KC_C)]
            wq_t = [wq_sb[:, c * 128:(c + 1) * 128] for c in range(KC_X)]
            wv_t = [wv_sb[:, c * 128:(c + 1) * 128] for c in range(KC_C)]
            wo_t = [wo_sb[:, m * 128:(m + 1) * 128] for m in range(MO)]

            # ---- persistent activation tiles ----
            ktA = kt_p.tile([128, NKV], BF16, name="ktA")
            ktB = kt_p.tile([128, NKV], BF16, name="ktB")
            nc.any.memset(ktA[64:128, :], 0.0)
            nc.any.memset(ktB[0:64, :], 0.0)
            qt = qt_p.tile([128, NQ], BF16, name="qt")
            # vaug: one tile, per-kv-chunk layout [65 for head A | 128 for B].
            # A = 64 V + ones col -> attn_A on psum partitions 0:64, denom_A
            # on 64. B = [zeros:32 | ones | zeros:31 | 64 V] -> attn_B lands
            # on psum partitions 64:128 and denom_B on partition 32, so every
            # SBUF operand pair in the normalize path shares a base partition.
            WB = 128
            WC = 65 + WB  # 193 per chunk
            va = vaug_p.tile([128, NKC * WC + 65], BF16, name="va")
            va3 = va[:, 0:NKC * WC].rearrange("p (g c) -> p g c", c=WC)
            nc.any.memset(va3[:, :, 64:65], 1.0)       # A ones col
            nc.any.memset(va3[:, :, 65:65 + 32], 0.0)  # B pad
            nc.any.memset(va3[:, :, 65 + 32:65 + 33], 1.0)  # B ones col
            nc.any.memset(va3[:, :, 65 + 33:65 + 64], 0.0)  # B pad

            # ---- K projection. Groups 0/1 (kv 0:1024) run contraction-outer
            # in the not-yet-used S psum tile, pipelined with ctx half-0
            # arrival. Groups 2/3 run later inside qc0's loop (ctx half-1).
            def emit_kproj_half0():
                pk01 = s_ps.tile([128, 1024], F32, tag="s0", name="pk01")
                kps = [pk01[:, 0:512], pk01[:, 512:1024]]
                for c in range(KC_C):
                    for n in range(2):
                        nc.tensor.matmul(
                            kps[n], wk_t[c], ctx_t[c][:, n * 512:(n + 1) * 512],
                            start=(c == 0), stop=(c == KC_C - 1))
                for n in range(2):
                    ns = slice(n * 512, (n + 1) * 512)
                    nc.vector.tensor_copy(ktA[0:64, ns], kps[n][0:64, :])
                    nc.vector.tensor_copy(ktB[64:128, ns], kps[n][64:128, :])

            def emit_kproj_group(n):
                ps = acc_ps.tile([128, 512], F32, tag="acc", name=f"pk{n}")
                for c in range(KC_C):
                    nc.tensor.matmul(
                        ps[:], wk_t[c], ctx_t[c][:, n * 512:(n + 1) * 512],
                        start=(c == 0), stop=(c == KC_C - 1))
                ns = slice(n * 512, (n + 1) * 512)
                nc.vector.tensor_copy(ktA[0:64, ns], ps[0:64, :])
                nc.vector.tensor_copy(ktB[64:128, ns], ps[64:128, :])

            def emit_qproj(n):
                ps = acc_ps.tile([128, 512], F32, tag="acc", name=f"pq{n}")
                for c in range(KC_X):
                    nc.tensor.matmul(
                        ps[:], wq_t[c], xt_t[c][:, n * 512:(n + 1) * 512],
                        start=(c == 0), stop=(c == KC_X - 1))
                nc.vector.tensor_copy(qt[:, n * 512:(n + 1) * 512], ps[:])

            # ---- V projection, 4 kv chunks per psum tile (interleaved into
            # qc 0); two strided copies evacuate all 4 chunks x both heads ----
            def emit_v4(g):
                ps = acc_ps.tile([128, 512], F32, tag="acc", name=f"pv{g}")
                for j in range(4):  # chunk-major: one open group per bank
                    kvc = g * 4 + j
                    for c in range(KC_C):
                        nc.tensor.matmul(
                            ps[:, j * 128:(j + 1) * 128],
                            ctx_t[c][:, kvc * 128:(kvc + 1) * 128], wv_t[c],
                            start=(c == 0), stop=(c == KC_C - 1))
                dst = va[:, g * 4 * WC:(g + 1) * 4 * WC].rearrange(
                    "p (c r) -> p c r", r=WC)
                src = ps[:].rearrange("p (c r) -> p c r", r=128)
                nc.vector.tensor_copy(dst[:, :, 0:64], src[:, :, 0:64])
                nc.vector.tensor_copy(dst[:, :, 129:193], src[:, :, 64:128])

            # ---- attention ----
            kt_h = {0: ktA, 1: ktB}
            psa = {}  # (head) -> live attn psum tile
            psa_sb = {}  # (head) -> SBUF copy of attn + denom
            p_ts = {}  # (head, pair) -> P tile
            at_tiles = [None] * NQC

            def emit_qk(h, qc, p):
                ps_s = s_ps.tile([128, 1024], F32, tag=f"s{h}", name=f"s{h}_{qc}_{p}")
                for j in range(2):
                    kvc = p * 2 + j
                    nc.tensor.matmul(
                        ps_s[:, j * 512:(j + 1) * 512],
                        kt_h[h][:, kvc * 128:(kvc + 1) * 128],
                        qt[:, qc * 512:(qc + 1) * 512],
                        start=True, stop=True)
                p_t = p_p.tile([128, 1024], BF16, tag=f"p{h}", name=f"p{h}_{qc}_{p}")
                nc.scalar.activation(p_t[:], ps_s[:],
                                     mybir.ActivationFunctionType.Exp, scale=SCALE)
                p_ts[(h, p)] = p_t

            def emit_pv(h, qc, p):
                w = 65 if h == 0 else WB
                off = 0 if h == 0 else 65
                if p == 0:
                    psa[h] = attn_ps.tile([w, 512], F32, tag=f"a{h}",
                                          name=f"a{h}_{qc}")
                for j in range(2):
                    kvc = p * 2 + j
                    nc.tensor.matmul(
                        psa[h][:], va[:, kvc * WC + off:kvc * WC + off + w],
                        p_ts[(h, p)][:, j * 512:(j + 1) * 512],
                        start=(kvc == 0), stop=(kvc == NKC - 1))

            # B attn rows: psum partitions 64:128; B denom: partition 32.
            A_DEN, B_DEN = 64, 32

            # normalize pipeline, spread across the next qc's pair iters:
            # evac attn psum -> SBUF bf16 (frees the psum banks for the next
            # qc's PV immediately), then Ln/Exp reciprocal rows, broadcast
            # matmul, SBUF-only multiply, output projection.
            def emit_psevac(qc):
                tA = at_p.tile([65, 512], BF16, tag="psA", name=f"psA{qc}")
                nc.vector.tensor_copy(tA[:], psa[0][:])
                psa_sb[0] = tA
                tB = at_p.tile([128, 512], BF16, tag="psB", name=f"psB{qc}")
                nc.vector.tensor_copy(tB[64:128, :], psa[1][64:128, :])
                nc.vector.tensor_copy(tB[32:33, :], psa[1][32:33, :])
                psa_sb[1] = tB

            def emit_recip(qc, h):
                qs = slice(qc * 512, (qc + 1) * 512)
                r = A_DEN if h == 0 else B_DEN
                nc.scalar.activation(scr[r:r + 1, :], psa_sb[h][r:r + 1, :],
                                     mybir.ActivationFunctionType.Ln,
                                     bias=zbias[r:r + 1, :])
                nc.scalar.activation(rec[r:r + 1, qs], scr[r:r + 1, :],
                                     mybir.ActivationFunctionType.Exp,
                                     bias=zbias[r:r + 1, :], scale=-1.0)

            def emit_bcast(qc):
                qs = slice(qc * 512, (qc + 1) * 512)
                ps_b = acc_ps.tile([128, 512], F32, tag="acc", name=f"bc{qc}")
                nc.tensor.matmul(ps_b[:], e_tile[:], rec[:, qs],
                                 start=True, stop=True)
                bc_sb = at_p.tile([128, 512], BF16, tag="bc", name=f"bcs{qc}")
                nc.vector.tensor_copy(bc_sb[:], ps_b[:])
                return bc_sb

            def emit_atmult(qc, bc_sb):
                a_t = at_p.tile([128, 512], BF16, tag="at", name=f"at{qc}")
                nc.vector.tensor_tensor(a_t[0:64, :], psa_sb[0][0:64, :],
                                        bc_sb[0:64, :], op=mybir.AluOpType.mult)
                nc.vector.tensor_tensor(a_t[64:128, :], psa_sb[1][64:128, :],
                                        bc_sb[64:128, :], op=mybir.AluOpType.mult)
                at_tiles[qc] = a_t

            def emit_oproj(qc):
                for m in range(MO):
                    ps = acc_ps.tile([128, 512], F32, tag="acc", name=f"o{qc}_{m}")
                    nc.tensor.matmul(ps[:], wo_t[m], at_tiles[qc][:],
                                     start=True, stop=True)
                    o_sb = out_p.tile([128, 512], F32, tag="osb")
                    nc.vector.tensor_copy(o_sb[:], ps[:])
                    nc.sync.dma_start(
                        ot[m * 128:(m + 1) * 128, qc * 512:(qc + 1) * 512], o_sb[:])

            # K proj half 0 pipelines with ctx arrival; Q group 0 follows;
            # K groups 2/3, remaining Q groups, and all V chunks interleave
            # into qc 0's ACT-paced gaps.
            emit_kproj_half0()
            emit_qproj(0)
            pend_bc = None
            for qc in range(NQC):
                for p in range(NPAIR + 1):
                    if p < NPAIR:
                        emit_qk(0, qc, p)
                        emit_qk(1, qc, p)
                    if qc == 0 and p == 0:
                        # gate wv behind the first exp so the scheduler cannot
                        # front-load V-proj ahead of the attention pipeline
                        nc.gpsimd.tensor_copy(wv_sb[0:1, 0:1],
                                              p_ts[(0, 0)][0:1, 0:1])
                        nc.sync.dma_start(wv_sb[:], wv[:, :])
                    if qc == 0:
                        if p in (0, 2, 4, 6):
                            emit_v4(p // 2)
                        if p == 3:
                            emit_kproj_group(2)
                            emit_kproj_group(3)
                        if p in (4, 5, 6):
                            emit_qproj(p - 3)
                    if qc > 0:
                        # spread the previous qc's normalize chain so no
                        # single engine sees a block of serial work
                        if p == 0:
                            emit_psevac(qc - 1)
                            emit_recip(qc - 1, 0)
                        elif p == 1:
                            emit_recip(qc - 1, 1)
                        elif p == 2:
                            pend_bc = emit_bcast(qc - 1)
                        elif p == 3:
                            emit_atmult(qc - 1, pend_bc)
                        elif p == 4:
                            emit_oproj(qc - 1)
                    if p >= 1:
                        emit_pv(0, qc, p - 1)
                        emit_pv(1, qc, p - 1)
            emit_psevac(NQC - 1)
            emit_recip(NQC - 1, 0)
            emit_recip(NQC - 1, 1)
            pend_bc = emit_bcast(NQC - 1)
            emit_atmult(NQC - 1, pend_bc)
            emit_oproj(NQC - 1)

    nc.finalize()
    return nc


def _bf16(a):
    return np.ascontiguousarray(a).astype(NP_BF16)


def run_spmd(inputs, trace=False):
    if "nc" not in _CACHE:
        _CACHE["nc"] = _build_nc()
    nc = _CACHE["nc"]

    x = np.asarray(inputs["x"], dtype=np.float32)
    context = np.asarray(inputs["context"], dtype=np.float32)
    wq_f = np.asarray(inputs["Wq"], np.float32)
    wk_f = np.asarray(inputs["Wk"], np.float32)
    wv_f = np.asarray(inputs["Wv"], np.float32)
    wo_f = np.asarray(inputs["Wo"], np.float32)
    bo_f = np.asarray(inputs["bo"], np.float32)

    def pack(w):
        # [K*128, 128] -> [128, K*128]: row p holds chunk-c columns side by
        # side, so one 128-row DMA carries all contraction chunks
        k = w.shape[0] // 128
        return _bf16(w.reshape(k, 128, 128).transpose(1, 0, 2).reshape(128, k * 128))

    xt_b = [_bf16(x[b].T) for b in range(B)]
    ctxt_b = [_bf16(context[b].T) for b in range(B)]
    in_maps = []
    for c in range(N_CORES):
        b, hp = c // 4, c % 4
        cs = slice(hp * 128, (hp + 1) * 128)
        in_maps.append({
            "xt": xt_b[b], "ctxt": ctxt_b[b],
            "wq": pack(wq_f[:, cs]), "wk": pack(wk_f[:, cs]),
            "wv": pack(wv_f[:, cs]), "wo": _bf16(wo_f[cs, :]),
        })

    res = run_bass_kernel_spmd(nc, in_maps, core_ids=list(range(N_CORES)),
                               trace=trace)
    out = np.empty((B, NQ, DI), dtype=np.float32)
    for b in range(B):
        acc = res.results[b * 4]["ot"].astype(np.float32)
        for hp in range(1, 4):
            acc = acc + res.results[b * 4 + hp]["ot"]
        out[b] = acc.T + bo_f[None, :]
    return out, res


def kernel(**inputs):
    out, _ = run_spmd(inputs, trace=False)
    return out


# revision 53
# speedup vs baseline: 1.1049x; 1.1049x over previous
"""Trainium2 Bass kernel for nn_CrossAttention (B=2, Nq=Nk=2048, H=8, Dh=64,
Dx=512, Dctx=768).

Sharding: (batch, head-pair) across 8 cores — core c = (batch c//4, head pair
c%4). Each core computes K/V/Q projections for its 2 heads only (no duplicated
projection work), full attention over all 2048 queries x 2048 keys for those
heads, and a PARTIAL output projection (rows hp*128:(hp+1)*128 of Wo). The
host sums the 4 partial outputs per batch at gather time (row-parallel Wo).

Every matmul is emitted in 128x128 PE tile mode (stationary partition dim and
moving partition dim are always 128) so the PE never switches tiling modes:
mode switches drain the array and hold the clock at the 1.2GHz mid p-state.
QK keeps 128-mode by zero-padding K along the contraction (dh) axis: ktA has
K_A^T in partitions 0:64 and zeros in 64:128; ktB the reverse. The shared
moving operand qt holds Q_A^T on partitions 0:64 and Q_B^T on 64:128.

Softmax runs without max-subtraction (scores ~N(0,1)); denominators come from
a ones-column in the augmented V stationary (head A: [64 V | ones] -> attn on
psum partitions 0:64, denom on 64; head B: [zeros | ones@32 | zeros | 64 V]
-> attn on partitions 64:128, denom on 32, keeping every operand pair of the
normalize path base-partition aligned). Reciprocal is exp(-ln(d)) on the
scalar engine (one pinned activation table holds both funcs); the reciprocal
rows are broadcast across partitions with a 128-mode matmul against a
constant indicator tile, then applied on the vector engine in SBUF/bf16.

DMA notes: wall time scales with partition-row descriptors, so weights are
host-packed into wide-row tiles and the bias is added on the host at gather;
ctx streams in kv-halves so K-proj/attention start after half the bytes;
non-critical transfers are issue-gated (tiny WAW copies) so the scheduler
cannot let them steal bandwidth or front-run the attention pipeline.
"""

import sys

sys.path.insert(0, "/opt/trn_rl_repo")

import numpy as np
import ml_dtypes

import concourse.bacc as bacc
import concourse.mybir as mybir
import concourse.tile as tile
from concourse.bass_utils import run_bass_kernel_spmd
from contextlib import ExitStack

F32 = mybir.dt.float32
BF16 = mybir.dt.bfloat16
NP_BF16 = np.dtype(ml_dtypes.bfloat16)

B = 2
NQ = 2048
NKV = 2048
DX = 512
DC = 768
DI = 512
NH = 8
DH = 64
N_CORES = 8

KC_X = DX // 128  # 4 contraction chunks for x
KC_C = DC // 128  # 6 contraction chunks for context
MO = DI // 128  # 4 output-row chunks
NKC = NKV // 128  # 16 kv chunks
NQC = NQ // 512  # 4 q chunks of 512
NPAIR = NKC // 2  # 8 kv chunk-pairs per q chunk
SCALE = DH ** -0.5

_CACHE = {}


def _build_nc():
    nc = bacc.Bacc("TRN2", target_bir_lowering=False, debug=False, num_devices=N_CORES)

    # weights arrive host-packed into wide-row tiles (one DMA descriptor per
    # partition row is the DMA cost unit, so [128, K*128] beats K x [128,128])
    xt = nc.declare_dram_parameter("xt", [DX, NQ], BF16, isOutput=False)
    ctxt = nc.declare_dram_parameter("ctxt", [DC, NKV], BF16, isOutput=False)
    wq = nc.declare_dram_parameter("wq", [128, KC_X * 128], BF16, isOutput=False)
    wk = nc.declare_dram_parameter("wk", [128, KC_C * 128], BF16, isOutput=False)
    wv = nc.declare_dram_parameter("wv", [128, KC_C * 128], BF16, isOutput=False)
    wo = nc.declare_dram_parameter("wo", [128, DI], BF16, isOutput=False)
    ot = nc.declare_dram_parameter("ot", [DI, NQ], F32, isOutput=True)

    with tile.TileContext(nc) as tc:
        with ExitStack() as ctx:
            # ---- SBUF pools ----
            const_p = ctx.enter_context(tc.tile_pool(name="const", bufs=1))
            w_p = ctx.enter_context(tc.tile_pool(name="weights", bufs=1))
            ctx_p = ctx.enter_context(tc.tile_pool(name="ctxt", bufs=1))
            xt_p = ctx.enter_context(tc.tile_pool(name="xt", bufs=1))
            kt_p = ctx.enter_context(tc.tile_pool(name="kt", bufs=1))
            qt_p = ctx.enter_context(tc.tile_pool(name="qt", bufs=1))
            vaug_p = ctx.enter_context(tc.tile_pool(name="vaug", bufs=1))
            p_p = ctx.enter_context(tc.tile_pool(name="pp", bufs=3))
            at_p = ctx.enter_context(tc.tile_pool(name="at", bufs=2))
            rec_p = ctx.enter_context(tc.tile_pool(name="rec", bufs=1))
            out_p = ctx.enter_context(tc.tile_pool(name="outsb", bufs=4))
            # ---- PSUM pools: 2 + 4 + 2 = 8 banks exactly ----
            acc_ps = ctx.enter_context(tc.tile_pool(name="acc_ps", bufs=2, space="PSUM"))
            s_ps = ctx.enter_context(tc.tile_pool(name="s_ps", bufs=1, space="PSUM"))
            attn_ps = ctx.enter_context(tc.tile_pool(name="attn_ps", bufs=1, space="PSUM"))

            # Pin the activation table to natural_log_exp_and_others (id 6):
            # it serves both Exp and Ln, so the table-load pass has no reason
            # to ping-pong between exp_and_others and natural_log (13 reloads
            # x 1283ns on the bottleneck scalar engine otherwise).
            nc.scalar.add_instruction(
                mybir.InstLoadActFuncSet(
                    name=nc.get_next_instruction_name(),
                    act_func_set_id=6, ins=[], outs=[]))

            # ---- constants ----
            # e_tile: bcast selector. col c<64 -> 1.0 at partition 64 (rec_A
            # row); col c>=64 -> 1.0 at partition 32 (rec_B row). Partition
            # bases must be 32-aligned (BIR verifier).
            e_tile = const_p.tile([128, 128], BF16)
            nc.any.memset(e_tile[:], 0.0)
            nc.any.memset(e_tile[64:65, 0:64], 1.0)
            nc.any.memset(e_tile[32:33, 64:128], 1.0)
            zbias = const_p.tile([128, 1], F32)
            nc.any.memset(zbias[:], 0.0)

            # rec: reciprocal rows (bf16), zeroed once; partitions 64/96 get
            # 1/d per qc column slice. All other rows stay 0 so the bcast
            # matmul's unused partitions contribute nothing (no NaN*0 risk).
            rec = rec_p.tile([128, NQ], BF16, name="rec")
            nc.any.memset(rec[:], 0.0)
            scr = rec_p.tile([128, 512], F32, name="lnscr")

            # ---- DMA inputs ----
            # DMA wall time scales with descriptor (partition-row) count, so:
            # packed weights (128 rows each), whole ctx/x tiles (4KB rows),
            # no bias transfer (host adds bo at gather). K-proj operands lead.
            wk_sb = w_p.tile([128, KC_C * 128], BF16, name="wk_sb")
            nc.sync.dma_start(wk_sb[:], wk[:, :])
            wq_sb = w_p.tile([128, KC_X * 128], BF16, name="wq_sb")
            nc.sync.dma_start(wq_sb[:], wq[:, :])
            ctx_t = [ctx_p.tile([128, NKV], BF16, tag=f"ctx{c}", name=f"ctx{c}")
                     for c in range(KC_C)]
            xt_t = [xt_p.tile([128, NQ], BF16, tag=f"xt{c}", name=f"xt{c}")
                    for c in range(KC_X)]
            wv_sb = w_p.tile([128, KC_C * 128], BF16, name="wv_sb")
            wo_sb = w_p.tile([128, DI], BF16, name="wo_sb")
            # ctx goes in kv-halves: K-proj groups 0/1 (and attention pairs
            # 0..3) only need kv 0:1024 of every feature chunk, so attention
            # starts after half the ctx bytes. xt0/wv ride between the
            # halves; the rest is issue-gated behind ctx (see below) so it
            # cannot steal bandwidth from the critical stream.
            # big transfers are split by partition halves/quarters: same
            # descriptor count, but they spread across queues (and the tile
            # scheduler's per-queue DMA model then sees them land early, so
            # it doesn't deprioritize the consumers)
            nc.sync.dma_start(ctx_t[0][:], ctxt[0:128, :])
            nc.sync.dma_start(xt_t[0][:], xt[0:128, :])
            for c in range(1, KC_C):
                nc.sync.dma_start(ctx_t[c][:], ctxt[c * 128:(c + 1) * 128, :])
            nc.sync.dma_start(wv_sb[:], wv[:, :])
            for c in range(1, KC_X):
                nc.sync.dma_start(xt_t[c][:], xt[c * 128:(c + 1) * 128, :])
            nc.sync.dma_start(wo_sb[:], wo[:, :])
            wk_t = [wk_sb[:, c * 128:(c + 1) * 128] for c in range(KC_C)]
            wq_t = [wq_sb[:, c * 128:(c + 1) * 128] for c in range(KC_X)]
            wv_t = [wv_sb[:, c * 128:(c + 1) * 128] for c in range(KC_C)]
            wo_t = [wo_sb[:, m * 128:(m + 1) * 128] for m in range(MO)]

            # ---- persistent activation tiles ----
            ktA = kt_p.tile([128, NKV], BF16, name="ktA")
            ktB = kt_p.tile([128, NKV], BF16, name="ktB")
            nc.any.memset(ktA[64:128, :], 0.0)
            nc.any.memset(ktB[0:64, :], 0.0)
            qt = qt_p.tile([128, NQ], BF16, name="qt")
            # vaug: one tile, per-kv-chunk layout [65 for head A | 128 for B].
            # A = 64 V + ones col -> attn_A on psum partitions 0:64, denom_A
            # on 64. B = [zeros:32 | ones | zeros:31 | 64 V] -> attn_B lands
            # on psum partitions 64:128 and denom_B on partition 32, so every
            # SBUF operand pair in the normalize path shares a base partition.
            WB = 128
            WC = 65 + WB  # 193 per chunk
            va = vaug_p.tile([128, NKC * WC + 65], BF16, name="va")
            va3 = va[:, 0:NKC * WC].rearrange("p (g c) -> p g c", c=WC)
            nc.any.memset(va3[:, :, 64:65], 1.0)       # A ones col
            nc.any.memset(va3[:, :, 65:65 + 32], 0.0)  # B pad
            nc.any.memset(va3[:, :, 65 + 32:65 + 33], 1.0)  # B ones col
            nc.any.memset(va3[:, :, 65 + 33:65 + 64], 0.0)  # B pad

            # ---- K projection, contraction-outer so each matmul fires as
            # its ctx chunk lands (all 4 kv groups accumulate in parallel in
            # the not-yet-used S psum tiles, one group per bank) ----
            def emit_kproj_all():
                pk01 = s_ps.tile([128, 1024], F32, tag="s0", name="pk01")
                pk23 = s_ps.tile([128, 1024], F32, tag="s1", name="pk23")
                kps = [pk01[:, 0:512], pk01[:, 512:1024],
                       pk23[:, 0:512], pk23[:, 512:1024]]
                for c in range(KC_C):
                    for n in range(4):
                        nc.tensor.matmul(
                            kps[n], wk_t[c], ctx_t[c][:, n * 512:(n + 1) * 512],
                            start=(c == 0), stop=(c == KC_C - 1))
                for n in range(4):
                    ns = slice(n * 512, (n + 1) * 512)
                    nc.vector.tensor_copy(ktA[0:64, ns], kps[n][0:64, :])
                    nc.vector.tensor_copy(ktB[64:128, ns], kps[n][64:128, :])

            def emit_qproj(n):
                ps = acc_ps.tile([128, 512], F32, tag="acc", name=f"pq{n}")
                for c in range(KC_X):
                    nc.tensor.matmul(
                        ps[:], wq_t[c], xt_t[c][:, n * 512:(n + 1) * 512],
                        start=(c == 0), stop=(c == KC_X - 1))
                nc.vector.tensor_copy(qt[:, n * 512:(n + 1) * 512], ps[:])

            # ---- V projection, 4 kv chunks per psum tile (interleaved into
            # qc 0); two strided copies evacuate all 4 chunks x both heads ----
            def emit_v4(g):
                ps = acc_ps.tile([128, 512], F32, tag="acc", name=f"pv{g}")
                for j in range(4):  # chunk-major: one open group per bank
                    kvc = g * 4 + j
                    for c in range(KC_C):
                        nc.tensor.matmul(
                            ps[:, j * 128:(j + 1) * 128],
                            ctx_t[c][:, kvc * 128:(kvc + 1) * 128], wv_t[c],
                            start=(c == 0), stop=(c == KC_C - 1))
                dst = va[:, g * 4 * WC:(g + 1) * 4 * WC].rearrange(
                    "p (c r) -> p c r", r=WC)
                src = ps[:].rearrange("p (c r) -> p c r", r=128)
                nc.vector.tensor_copy(dst[:, :, 0:64], src[:, :, 0:64])
                nc.vector.tensor_copy(dst[:, :, 129:193], src[:, :, 64:128])

            # ---- attention ----
            kt_h = {0: ktA, 1: ktB}
            psa = {}  # (head) -> live attn psum tile
            psa_sb = {}  # (head) -> SBUF copy of attn + denom
            p_ts = {}  # (head, pair) -> P tile
            at_tiles = [None] * NQC

            def emit_qk(h, qc, p):
                ps_s = s_ps.tile([128, 1024], F32, tag=f"s{h}", name=f"s{h}_{qc}_{p}")
                for j in range(2):
                    kvc = p * 2 + j
                    nc.tensor.matmul(
                        ps_s[:, j * 512:(j + 1) * 512],
                        kt_h[h][:, kvc * 128:(kvc + 1) * 128],
                        qt[:, qc * 512:(qc + 1) * 512],
                        start=True, stop=True)
                p_t = p_p.tile([128, 1024], BF16, tag=f"p{h}", name=f"p{h}_{qc}_{p}")
                nc.scalar.activation(p_t[:], ps_s[:],
                                     mybir.ActivationFunctionType.Exp, scale=SCALE)
                p_ts[(h, p)] = p_t

            def emit_pv(h, qc, p):
                w = 65 if h == 0 else WB
                off = 0 if h == 0 else 65
                if p == 0:
                    psa[h] = attn_ps.tile([w, 512], F32, tag=f"a{h}",
                                          name=f"a{h}_{qc}")
                for j in range(2):
                    kvc = p * 2 + j
                    nc.tensor.matmul(
                        psa[h][:], va[:, kvc * WC + off:kvc * WC + off + w],
                        p_ts[(h, p)][:, j * 512:(j + 1) * 512],
                        start=(kvc == 0), stop=(kvc == NKC - 1))

            # B attn rows: psum partitions 64:128; B denom: partition 32.
            A_DEN, B_DEN = 64, 32

            # normalize pipeline, spread across the next qc's pair iters:
            # evac attn psum -> SBUF bf16 (frees the psum banks for the next
            # qc's PV immediately), then Ln/Exp reciprocal rows, broadcast
            # matmul, SBUF-only multiply, output projection.
            def emit_psevac(qc):
                tA = at_p.tile([65, 512], BF16, tag="psA", name=f"psA{qc}")
                nc.vector.tensor_copy(tA[:], psa[0][:])
                psa_sb[0] = tA
                tB = at_p.tile([128, 512], BF16, tag="psB", name=f"psB{qc}")
                nc.vector.tensor_copy(tB[64:128, :], psa[1][64:128, :])
                nc.vector.tensor_copy(tB[32:33, :], psa[1][32:33, :])
                psa_sb[1] = tB

            def emit_recip(qc, h):
                qs = slice(qc * 512, (qc + 1) * 512)
                r = A_DEN if h == 0 else B_DEN
                nc.scalar.activation(scr[r:r + 1, :], psa_sb[h][r:r + 1, :],
                                     mybir.ActivationFunctionType.Ln,
                                     bias=zbias[r:r + 1, :])
                nc.scalar.activation(rec[r:r + 1, qs], scr[r:r + 1, :],
                                     mybir.ActivationFunctionType.Exp,
                                     bias=zbias[r:r + 1, :], scale=-1.0)

            def emit_bcast(qc):
                qs = slice(qc * 512, (qc + 1) * 512)
                ps_b = acc_ps.tile([128, 512], F32, tag="acc", name=f"bc{qc}")
                nc.tensor.matmul(ps_b[:], e_tile[:], rec[:, qs],
                                 start=True, stop=True)
                bc_sb = at_p.tile([128, 512], BF16, tag="bc", name=f"bcs{qc}")
                nc.vector.tensor_copy(bc_sb[:], ps_b[:])
                return bc_sb

            def emit_atmult(qc, bc_sb):
                a_t = at_p.tile([128, 512], BF16, tag="at", name=f"at{qc}")
                nc.vector.tensor_tensor(a_t[0:64, :], psa_sb[0][0:64, :],
                                        bc_sb[0:64, :], op=mybir.AluOpType.mult)
                nc.vector.tensor_tensor(a_t[64:128, :], psa_sb[1][64:128, :],
                                        bc_sb[64:128, :], op=mybir.AluOpType.mult)
                at_tiles[qc] = a_t

            def emit_oproj(qc):
                for m in range(MO):
                    ps = acc_ps.tile([128, 512], F32, tag="acc", name=f"o{qc}_{m}")
                    nc.tensor.matmul(ps[:], wo_t[m], at_tiles[qc][:],
                                     start=True, stop=True)
                    o_sb = out_p.tile([128, 512], F32, tag="osb")
                    nc.vector.tensor_copy(o_sb[:], ps[:])
                    nc.sync.dma_start(
                        ot[m * 128:(m + 1) * 128, qc * 512:(qc + 1) * 512], o_sb[:])

            # K proj pipelines with ctx arrival; Q group 0 follows; remaining
            # Q groups and all V chunks interleave into qc 0's ACT-paced gaps.
            emit_kproj_all()
            emit_qproj(0)
            pend_bc = None
            for qc in range(NQC):
                for p in range(NPAIR + 1):
                    if p < NPAIR:
                        emit_qk(0, qc, p)
                        emit_qk(1, qc, p)
                    if qc == 0:
                        if p in (0, 2, 4, 6):
                            emit_v4(p // 2)
                        if p in (1, 3, 5):
                            emit_qproj((p - 1) // 2 + 1)
                    if qc > 0:
                        # spread the previous qc's normalize chain so no
                        # single engine sees a block of serial work
                        if p == 0:
                            emit_psevac(qc - 1)
                            emit_recip(qc - 1, 0)
                        elif p == 1:
                            emit_recip(qc - 1, 1)
                        elif p == 2:
                            pend_bc = emit_bcast(qc - 1)
                        elif p == 3:
                            emit_atmult(qc - 1, pend_bc)
                        elif p == 4:
                            emit_oproj(qc - 1)
                    if p >= 1:
                        emit_pv(0, qc, p - 1)
                        emit_pv(1, qc, p - 1)
            emit_psevac(NQC - 1)
            emit_recip(NQC - 1, 0)
            emit_recip(NQC - 1, 1)
            pend_bc = emit_bcast(NQC - 1)
            emit_atmult(NQC - 1, pend_bc)
            emit_oproj(NQC - 1)

    nc.finalize()
    return nc


def _bf16(a):
    return np.ascontiguousarray(a).astype(NP_BF16)


def run_spmd(inputs, trace=False):
    if "nc" not in _CACHE:
        _CACHE["nc"] = _build_nc()
    nc = _CACHE["nc"]

    x = np.asarray(inputs["x"], dtype=np.float32)
    context = np.asarray(inputs["context"], dtype=np.float32)
    wq_f = np.asarray(inputs["Wq"], np.float32)
    wk_f = np.asarray(inputs["Wk"], np.float32)
    wv_f = np.asarray(inputs["Wv"], np.float32)
    wo_f = np.asarray(inputs["Wo"], np.float32)
    bo_f = np.asarray(inputs["bo"], np.float32)

    def pack(w):
        # [K*128, 128] -> [128, K*128]: row p holds chunk-c columns side by
        # side, so one 128-row DMA carries all contraction chunks
        k = w.shape[0] // 128
        return _bf16(w.reshape(k, 128, 128).transpose(1, 0, 2).reshape(128, k * 128))

    xt_b = [_bf16(x[b].T) for b in range(B)]
    ctxt_b = [_bf16(context[b].T) for b in range(B)]
    in_maps = []
    for c in range(N_CORES):
        b, hp = c // 4, c % 4
        cs = slice(hp * 128, (hp + 1) * 128)
        in_maps.append({
            "xt": xt_b[b], "ctxt": ctxt_b[b],
            "wq": pack(wq_f[:, cs]), "wk": pack(wk_f[:, cs]),
            "wv": pack(wv_f[:, cs]), "wo": _bf16(wo_f[cs, :]),
        })

    res = run_bass_kernel_spmd(nc, in_maps, core_ids=list(range(N_CORES)),
                               trace=trace)
    out = np.empty((B, NQ, DI), dtype=np.float32)
    for b in range(B):
        acc = res.results[b * 4]["ot"].astype(np.float32)
        for hp in range(1, 4):
            acc = acc + res.results[b * 4 + hp]["ot"]
        out[b] = acc.T + bo_f[None, :]
    return out, res


def kernel(**inputs):
    out, _ = run_spmd(inputs, trace=False)
    return out
